# revision 9
# baseline (speedup 1.0000x reference)
"""Multi-head attention (B=2, S=2048, D=1024, H=16, causal) on 8 TRN2 NeuronCores.

Sharding: 8 cores = 2 batches x 4 head-groups (4 heads each).  Each core
computes the QKV projections for its head slice, causal attention for its 4
heads, and the partial output projection (input-dim slice of Wo).  The
all-reduce over head-groups happens at gather time on the host (sum of 4
partials per batch), which is the "all-reduce after the output projection"
of a tensor-parallel split.

Everything on device works in token-transposed layout ([feature, token]) so
no on-device transposes are needed:
  scores^T[kv, q] = K_projT_tile^T @ Q_projT   (K = dh = 64)
  P = exp(scores^T)  (no max subtraction needed: scores ~ N(0,1), |s| < ~7)
  out^T[dh(+1), q] = [V | ones]^T @ P          (ones column -> softmax denom)
  partial^T[dmodel, tok] = WoT_slice^T @ attn_out^T

Perf notes:
  - causal trimming: score matmul / exp / AV only cover valid q columns of
    diagonal kv-tiles; the per-tile mask multiply shrinks to one 128x128
    causal block.
  - score matmuls have K=64 (half the PE rows): odd kv-tiles are issued at
    tile rows 64-127 (via partition-swapped copies of Q/K projections) so
    adjacent score matmuls run concurrently in the PE array.
  - O-projection accumulates in two half-groups (heads 0+1, heads 2+3) so
    the heads-0/1 half runs on the PE while heads 2/3 attention is still
    ACT-bound; the halves are summed in the output copy.
"""

import math
import os

import numpy as np
import ml_dtypes

_BF16 = ml_dtypes.bfloat16

B, S, D = 2, 2048, 1024
H, DH = 16, 64
NCORES = 8
GRP = 4  # heads per core
KT = D // 128  # 8 k-tiles over d_model
NQ = 512  # q tile width (free dim of score tiles)
QTILES = S // NQ  # 4
KVTILES = S // 128  # 16

last_results = None

_programs = {}


def _build_program(causal: bool):
    import concourse.bass as bass
    import concourse.mybir as mybir
    import concourse.tile as tile
    from concourse import bacc

    f32 = mybir.dt.float32
    bf16 = mybir.dt.bfloat16
    Exp = mybir.ActivationFunctionType.Exp
    Copy = mybir.ActivationFunctionType.Copy

    nc = bacc.Bacc(
        "TRN2",
        target_bir_lowering=False,
        debug=False,
        enable_asserts=False,
        num_devices=NCORES,
    )

    qT = nc.dram_tensor("qT", [D, S], bf16, kind="ExternalInput").ap()
    kT = nc.dram_tensor("kT", [D, S], bf16, kind="ExternalInput").ap()
    vT = nc.dram_tensor("vT", [D, S], bf16, kind="ExternalInput").ap()
    wqT = nc.dram_tensor("wqT", [D, 256], bf16, kind="ExternalInput").ap()
    wkT = nc.dram_tensor("wkT", [D, 256], bf16, kind="ExternalInput").ap()
    wvT = nc.dram_tensor("wvT", [D, 256], bf16, kind="ExternalInput").ap()
    woT = nc.dram_tensor("woT", [256, D], bf16, kind="ExternalInput").ap()
    if not causal:
        maskT = nc.dram_tensor("maskT", [S, S], bf16, kind="ExternalInput").ap()
    out = nc.dram_tensor("out", [D, S], f32, kind="ExternalOutput").ap()

    with tile.TileContext(nc) as tc:
        with (
            tc.tile_pool(name="persist", bufs=1) as sb,
            tc.tile_pool(name="stream", bufs=8) as stream,
            tc.tile_pool(name="psum", bufs=1, space="PSUM") as psum,
            tc.tile_pool(name="p_sb", bufs=6) as pbuf,
            tc.tile_pool(name="r_sb", bufs=3) as rpool,
            tc.tile_pool(name="m_sb", bufs=4) as mpool,
            tc.tile_pool(name="o_sb", bufs=4) as opool,
        ):
            # ---- persistent SBUF tensors ----
            wq_sb = sb.tile([128, KT, 256], bf16)
            wk_sb = sb.tile([128, KT, 256], bf16)
            wv_sb = sb.tile([128, KT, 256], bf16)
            wo_sb = sb.tile([64, GRP, D], bf16)
            qproj = sb.tile([128, 2, S], bf16)
            kproj = sb.tile([128, 2, S], bf16)
            qprojS = sb.tile([128, 2, S], bf16)  # partition-halves swapped
            kprojS = sb.tile([128, 2, S], bf16)
            vproj = sb.tile([128, KVTILES, GRP, 66], bf16)
            attn = sb.tile([64, GRP, S], bf16)

            for kt in range(KT):
                nc.sync.dma_start(wq_sb[:, kt, :], wqT[128 * kt : 128 * kt + 128, :])
                nc.sync.dma_start(wk_sb[:, kt, :], wkT[128 * kt : 128 * kt + 128, :])
                nc.sync.dma_start(wv_sb[:, kt, :], wvT[128 * kt : 128 * kt + 128, :])
            for h in range(GRP):
                nc.sync.dma_start(wo_sb[:, h, :], woT[64 * h : 64 * h + 64, :])

            # ones columns at index 0 and 65 of vproj (V lands in cols 1..64)
            nc.gpsimd.memset(vproj[:], 1.0)

            if causal:
                # single 128x128 causal block: keep where q_local >= kv_local
                mask128 = sb.tile([128, 128], bf16)
                nc.gpsimd.memset(mask128[:], 1.0)
                nc.gpsimd.affine_select(
                    out=mask128[:],
                    in_=mask128[:],
                    compare_op=mybir.AluOpType.is_ge,
                    fill=0.0,
                    base=0,
                    pattern=[[1, 128]],
                    channel_multiplier=-1,
                )

            # ---- Q / K projections (transposed): proj^T[256, S] ----
            for w_sb, x_dram, proj in ((wq_sb, qT, qproj), (wk_sb, kT, kproj)):
                xt = []
                for kt in range(KT):
                    t = stream.tile([128, S], bf16, tag="xT")
                    nc.sync.dma_start(t[:], x_dram[128 * kt : 128 * kt + 128, :])
                    xt.append(t)
                for m2 in range(2):
                    for n in range(QTILES):
                        ps = psum.tile([128, NQ], f32, tag="ps", bufs=4)
                        for kt in range(KT):
                            nc.tensor.matmul(
                                ps[:],
                                w_sb[:, kt, 128 * m2 : 128 * m2 + 128],
                                xt[kt][:, NQ * n : NQ * n + NQ],
                                start=(kt == 0),
                                stop=(kt == KT - 1),
                            )
                        nc.vector.tensor_copy(proj[:, m2, NQ * n : NQ * n + NQ], ps[:])

            # swapped-partition copies (for row-packed score matmuls)
            nc.sync.dma_start(qprojS[0:64, :, :], qproj[64:128, :, :])
            nc.sync.dma_start(qprojS[64:128, :, :], qproj[0:64, :, :])
            nc.sync.dma_start(kprojS[0:64, :, :], kproj[64:128, :, :])
            nc.sync.dma_start(kprojS[64:128, :, :], kproj[0:64, :, :])

            # ---- V projection (normal layout): v_proj[tok, 256] ----
            vt = []
            for kt in range(KT):
                t = stream.tile([128, S], bf16, tag="xT")
                nc.sync.dma_start(t[:], vT[128 * kt : 128 * kt + 128, :])
                vt.append(t)
            for mt in range(KVTILES):
                ps = psum.tile([128, 256], f32, tag="ps", bufs=4)
                for kt in range(KT):
                    nc.tensor.matmul(
                        ps[:],
                        vt[kt][:, 128 * mt : 128 * mt + 128],
                        wv_sb[:, kt, :],
                        start=(kt == 0),
                        stop=(kt == KT - 1),
                    )
                nc.vector.tensor_copy(
                    vproj[:, mt, :, 1:65],
                    ps[:].rearrange("p (h d) -> p h d", h=GRP),
                )

            # ---- attention ----
            def attention_head(h):
                h2, hp = h // 2, 64 * (h % 2)
                hpS = 64 - hp  # partition base of head h in the swapped copies
                for j in range(QTILES):
                    av = psum.tile([65, NQ], f32, tag="av", bufs=3)
                    ktiles = 4 * j + 4 if causal else KVTILES
                    for t in range(ktiles):
                        # causal trim: diagonal kv-tile t covers q in [off, NQ)
                        d = t - 4 * j
                        off = 128 * d if (causal and d >= 0) else 0
                        w = NQ - off
                        if t % 2 == 0:
                            kp, qp, base = kproj, qproj, hp
                        else:
                            kp, qp, base = kprojS, qprojS, hpS
                        sp = psum.tile([128, NQ], f32, tag="ps", bufs=4)
                        nc.tensor.matmul(
                            sp[:, off:NQ],
                            kp[base : base + 64, h2, 128 * t : 128 * t + 128],
                            qp[base : base + 64, h2, NQ * j + off : NQ * j + NQ],
                            start=True,
                            stop=True,
                        )
                        p = pbuf.tile([128, NQ], bf16, tag="p")
                        nc.scalar.activation(p[:, off:NQ], sp[:, off:NQ], Exp)
                        if causal:
                            if d >= 0:
                                nc.vector.tensor_mul(
                                    p[:, off : off + 128],
                                    p[:, off : off + 128],
                                    mask128[:],
                                )
                        else:
                            mt_t = mpool.tile([128, NQ], bf16, tag="mt")
                            nc.sync.dma_start(
                                mt_t[:],
                                maskT[128 * t : 128 * t + 128, NQ * j : NQ * j + NQ],
                            )
                            nc.vector.tensor_mul(p[:], p[:], mt_t[:])
                        nc.tensor.matmul(
                            av[:, off:NQ],
                            vproj[:, t, h, 1:66],
                            p[:, off:NQ],
                            start=(t == 0),
                            stop=(t == ktiles - 1),
                        )
                    # normalize: attn[:, h, q] = av[0:64, q] / av[64, q]
                    # (single-row reciprocal is slow on DVE -> DMA-reshape the
                    #  512 sums to [128, 4], recip there, reshape to partition
                    #  0, gpsimd-broadcast to 64 partitions)
                    rs = rpool.tile([65, NQ], f32, tag="rs")
                    nc.vector.tensor_copy(rs[64:65, :], av[64:65, :])
                    rq = rpool.tile([128, 4], f32, tag="rq")
                    nc.sync.dma_start(rq[:], rs[64:65, :])
                    rqr = rpool.tile([128, 4], f32, tag="rqr")
                    nc.vector.reciprocal(rqr[:], rq[:])
                    rr = rpool.tile([1, NQ], f32, tag="rr")
                    nc.sync.dma_start(rr[:], rqr[:])
                    rb = rpool.tile([64, NQ], f32, tag="rb")
                    nc.gpsimd.partition_broadcast(rb[:], rr[0:1, :], channels=64)
                    nc.vector.tensor_mul(
                        attn[:, h, NQ * j : NQ * j + NQ], av[0:64, :], rb[:]
                    )

            attention_head(0)
            attention_head(1)

            # O-projection first half (heads 0,1) — fills the PE while heads
            # 2/3 attention is ACT-bound.  s01 copies go on ScalarE.
            s01s = {}
            for m in range(D // 128):
                for n in range(QTILES):
                    ps = psum.tile([128, NQ], f32, tag="ps", bufs=4)
                    for h in range(2):
                        nc.tensor.matmul(
                            ps[:],
                            wo_sb[:, h, 128 * m : 128 * m + 128],
                            attn[:, h, NQ * n : NQ * n + NQ],
                            start=(h == 0),
                            stop=(h == 1),
                        )
                    s01 = opool.tile([128, NQ], bf16, tag="s01", bufs=32)
                    nc.scalar.activation(s01[:], ps[:], Copy)
                    s01s[(m, n)] = s01

            attention_head(2)
            attention_head(3)

            for m in range(D // 128):
                for n in range(QTILES):
                    ps = psum.tile([128, NQ], f32, tag="ps", bufs=4)
                    for h in range(2, 4):
                        nc.tensor.matmul(
                            ps[:],
                            wo_sb[:, h, 128 * m : 128 * m + 128],
                            attn[:, h, NQ * n : NQ * n + NQ],
                            start=(h == 2),
                            stop=(h == 3),
                        )
                    ot = opool.tile([128, NQ], f32, tag="ot")
                    nc.vector.tensor_add(ot[:], ps[:], s01s[(m, n)][:])
                    nc.sync.dma_start(
                        out[128 * m : 128 * m + 128, NQ * n : NQ * n + NQ], ot[:]
                    )

    nc.compile()
    return nc


def _get_program(causal: bool):
    if causal not in _programs:
        _programs[causal] = _build_program(causal)
    return _programs[causal]


def kernel(query, key, value, mask, Wq, Wk, Wv, Wo):
    global last_results
    from concourse.bass_utils import run_bass_kernel_spmd

    query = np.asarray(query, dtype=np.float32)
    key = np.asarray(key, dtype=np.float32)
    value = np.asarray(value, dtype=np.float32)
    Wq = np.asarray(Wq, dtype=np.float32)
    Wk = np.asarray(Wk, dtype=np.float32)
    Wv = np.asarray(Wv, dtype=np.float32)
    Wo = np.asarray(Wo, dtype=np.float32)
    m2d = np.asarray(mask).reshape(S, S).astype(bool)

    causal = bool(np.array_equal(m2d, np.tril(np.ones((S, S), dtype=bool))))
    nc = _get_program(causal)

    scale = 1.0 / math.sqrt(DH)
    WqT = np.ascontiguousarray((Wq * scale).T).astype(_BF16)
    WkT = np.ascontiguousarray(Wk.T).astype(_BF16)
    WvT = np.ascontiguousarray(Wv.T).astype(_BF16)
    WoT = np.ascontiguousarray(Wo.T).astype(_BF16)
    xT = {
        "qT": [np.ascontiguousarray(query[b].T).astype(_BF16) for b in range(B)],
        "kT": [np.ascontiguousarray(key[b].T).astype(_BF16) for b in range(B)],
        "vT": [np.ascontiguousarray(value[b].T).astype(_BF16) for b in range(B)],
    }
    if not causal:
        maskT = np.ascontiguousarray(m2d.T).astype(_BF16)

    in_maps = []
    for c in range(NCORES):
        b, g = c // 4, c % 4
        sl = slice(256 * g, 256 * g + 256)
        im = {
            "qT": xT["qT"][b],
            "kT": xT["kT"][b],
            "vT": xT["vT"][b],
            "wqT": np.ascontiguousarray(WqT[:, sl]),
            "wkT": np.ascontiguousarray(WkT[:, sl]),
            "wvT": np.ascontiguousarray(WvT[:, sl]),
            "woT": np.ascontiguousarray(WoT[sl, :]),
        }
        if not causal:
            im["maskT"] = maskT
        in_maps.append(im)

    trace = os.environ.get("KERNEL_PROFILE", "") == "1"
    res = run_bass_kernel_spmd(nc, in_maps, list(range(NCORES)), trace=trace)
    last_results = res

    outp = np.empty((B, S, D), dtype=np.float32)
    for b in range(B):
        acc = res.results[4 * b]["out"].astype(np.float32)
        for g in range(1, 4):
            acc = acc + res.results[4 * b + g]["out"]
        outp[b] = acc.T
    return outp


# revision 17
# speedup vs baseline: 1.1096x; 1.1096x over previous
"""Multi-head attention (B=2, S=2048, D=1024, H=16, causal) on 8 TRN2 NeuronCores.

Sharding: 8 cores = 2 batches x 4 head-groups (4 heads each).  Each core
computes the QKV projections for its head slice, causal attention for its 4
heads, and the partial output projection (input-dim slice of Wo).  The
all-reduce over head-groups happens at gather time on the host (sum of 4
partials per batch), which is the "all-reduce after the output projection"
of a tensor-parallel split.

Everything on device works in token-transposed layout ([feature, token]) so
no on-device transposes are needed:
  scores^T[kv, q] = K_projT_tile^T @ Q_projT   (K = dh = 64)
  P = exp(scores^T)  (no max subtraction needed: scores ~ N(0,1), |s| < ~7)
  out^T[dh(+1), q] = [V | ones]^T @ P          (ones column -> softmax denom)
  partial^T[dmodel, tok] = WoT_slice^T @ attn_out^T

Perf notes:
  - causal trimming: score matmul / exp / AV only cover valid q columns of
    diagonal kv-tiles; the per-tile mask multiply shrinks to one 128x128
    causal block.
  - score matmuls have K=64 (half the PE rows): odd kv-tiles are issued at
    tile rows 64-127 (via partition-swapped copies of Q/K projections) so
    adjacent score matmuls run concurrently in the PE array.
  - O-projection accumulates in two half-groups (heads 0+1, heads 2+3) so
    the heads-0/1 half runs on the PE while heads 2/3 attention is still
    ACT-bound; the halves are summed in the output copy.
"""

import math
import os

import numpy as np
import ml_dtypes

_BF16 = ml_dtypes.bfloat16

B, S, D = 2, 2048, 1024
H, DH = 16, 64
NCORES = 8
GRP = 4  # heads per core
KT = D // 128  # 8 k-tiles over d_model
NQ = 512  # q tile width (free dim of score tiles)
QTILES = S // NQ  # 4
KVTILES = S // 128  # 16

last_results = None

_programs = {}


def _build_program(causal: bool):
    OPT_INLINE = os.environ.get("KOPT_INLINE", "1") == "1"
    OPT_OPACK = os.environ.get("KOPT_OPACK", "1") == "1"
    OPT_SCADJ = os.environ.get("KOPT_SCADJ", "1") == "1"

    import concourse.bass as bass
    import concourse.mybir as mybir
    import concourse.tile as tile
    from concourse import bacc

    f32 = mybir.dt.float32
    bf16 = mybir.dt.bfloat16
    Exp = mybir.ActivationFunctionType.Exp
    Copy = mybir.ActivationFunctionType.Copy

    nc = bacc.Bacc(
        "TRN2",
        target_bir_lowering=False,
        debug=False,
        enable_asserts=False,
        num_devices=NCORES,
    )

    qT = nc.dram_tensor("qT", [D, S], bf16, kind="ExternalInput").ap()
    kT = nc.dram_tensor("kT", [D, S], bf16, kind="ExternalInput").ap()
    vT = nc.dram_tensor("vT", [D, S], bf16, kind="ExternalInput").ap()
    wqT = nc.dram_tensor("wqT", [D, 256], bf16, kind="ExternalInput").ap()
    wkT = nc.dram_tensor("wkT", [D, 256], bf16, kind="ExternalInput").ap()
    wvT = nc.dram_tensor("wvT", [D, 256], bf16, kind="ExternalInput").ap()
    woT = nc.dram_tensor("woT", [256, D], bf16, kind="ExternalInput").ap()
    if not causal:
        maskT = nc.dram_tensor("maskT", [S, S], bf16, kind="ExternalInput").ap()
    out = nc.dram_tensor("out", [D, S], f32, kind="ExternalOutput").ap()

    with tile.TileContext(nc) as tc:
        with (
            tc.tile_pool(name="persist", bufs=1) as sb,
            tc.tile_pool(name="stream", bufs=8) as stream,
            tc.tile_pool(name="psum", bufs=1, space="PSUM") as psum,
            tc.tile_pool(name="p_sb", bufs=6) as pbuf,
            tc.tile_pool(name="r_sb", bufs=3) as rpool,
            tc.tile_pool(name="m_sb", bufs=4) as mpool,
            tc.tile_pool(name="o_sb", bufs=4) as opool,
        ):
            # ---- persistent SBUF tensors ----
            wq_sb = sb.tile([128, KT, 256], bf16)
            wk_sb = sb.tile([128, KT, 256], bf16)
            wv_sb = sb.tile([128, KT, 256], bf16)
            wo2 = sb.tile([128, 2, D], bf16)  # head h at rows 64*(h%2), chunk h//2
            qproj = sb.tile([128, 2, S], bf16)
            kproj = sb.tile([128, 2, S], bf16)
            qprojS = sb.tile([128, 2, S], bf16)  # partition-halves swapped
            kprojS = sb.tile([128, 2, S], bf16)
            vproj = sb.tile([128, KVTILES, GRP, 66], bf16)
            attn2 = sb.tile([128, 2, S], bf16)  # head h at rows 64*(h%2), chunk h//2

            for kt in range(KT):
                nc.sync.dma_start(wq_sb[:, kt, :], wqT[128 * kt : 128 * kt + 128, :])
                nc.sync.dma_start(wk_sb[:, kt, :], wkT[128 * kt : 128 * kt + 128, :])
                nc.sync.dma_start(wv_sb[:, kt, :], wvT[128 * kt : 128 * kt + 128, :])
            for h in range(GRP):
                base = 64 * (h % 2)
                nc.sync.dma_start(
                    wo2[base : base + 64, h // 2, :], woT[64 * h : 64 * h + 64, :]
                )

            # ones columns at index 0 and 65 of vproj (V lands in cols 1..64)
            nc.gpsimd.memset(vproj[:], 1.0)

            if causal:
                # single 128x128 causal block: keep where q_local >= kv_local
                mask128 = sb.tile([128, 128], bf16)
                nc.gpsimd.memset(mask128[:], 1.0)
                nc.gpsimd.affine_select(
                    out=mask128[:],
                    in_=mask128[:],
                    compare_op=mybir.AluOpType.is_ge,
                    fill=0.0,
                    base=0,
                    pattern=[[1, 128]],
                    channel_multiplier=-1,
                )

            # ---- Q / K projections (transposed): proj^T[256, S] ----
            for w_sb, x_dram, proj in ((wq_sb, qT, qproj), (wk_sb, kT, kproj)):
                xt = []
                for kt in range(KT):
                    t = stream.tile([128, S], bf16, tag="xT")
                    nc.sync.dma_start(t[:], x_dram[128 * kt : 128 * kt + 128, :])
                    xt.append(t)
                for m2 in range(2):
                    for n in range(QTILES):
                        ps = psum.tile([128, NQ], f32, tag="sc", bufs=3)
                        for kt in range(KT):
                            nc.tensor.matmul(
                                ps[:],
                                w_sb[:, kt, 128 * m2 : 128 * m2 + 128],
                                xt[kt][:, NQ * n : NQ * n + NQ],
                                start=(kt == 0),
                                stop=(kt == KT - 1),
                            )
                        nc.vector.tensor_copy(proj[:, m2, NQ * n : NQ * n + NQ], ps[:])

            # swapped-partition copies (for row-packed score matmuls)
            nc.sync.dma_start(qprojS[0:64, :, :], qproj[64:128, :, :])
            nc.sync.dma_start(qprojS[64:128, :, :], qproj[0:64, :, :])
            nc.sync.dma_start(kprojS[0:64, :, :], kproj[64:128, :, :])
            nc.sync.dma_start(kprojS[64:128, :, :], kproj[0:64, :, :])

            # ---- V projection (normal layout): v_proj[tok, 256] ----
            vt = []
            for kt in range(KT):
                t = stream.tile([128, S], bf16, tag="xT")
                nc.sync.dma_start(t[:], vT[128 * kt : 128 * kt + 128, :])
                vt.append(t)
            for mt in range(KVTILES):
                ps = psum.tile([128, 256], f32, tag="sc", bufs=3)
                for kt in range(KT):
                    nc.tensor.matmul(
                        ps[:],
                        vt[kt][:, 128 * mt : 128 * mt + 128],
                        wv_sb[:, kt, :],
                        start=(kt == 0),
                        stop=(kt == KT - 1),
                    )
                nc.vector.tensor_copy(
                    vproj[:, mt, :, 1:65],
                    ps[:].rearrange("p (h d) -> p h d", h=GRP),
                )

            # ---- attention ----
            def oproj_groups(n):
                # one token-tile worth of output projection; within each
                # accumulation group the 4 head-matmuls alternate row halves
                # (split-K row packing, ~2x on the PE)
                for m in range(D // 128):
                    ps = psum.tile([128, NQ], f32, tag="op", bufs=2)
                    # head pairs are stacked in partition halves -> K=128
                    # contracts two heads per matmul
                    for c2 in range(2):
                        nc.tensor.matmul(
                            ps[:],
                            wo2[:, c2, 128 * m : 128 * m + 128],
                            attn2[:, c2, NQ * n : NQ * n + NQ],
                            start=(c2 == 0),
                            stop=(c2 == 1),
                        )
                    ot = opool.tile([128, NQ], f32, tag="ot")
                    nc.vector.tensor_copy(ot[:], ps[:])
                    nc.sync.dma_start(
                        out[128 * m : 128 * m + 128, NQ * n : NQ * n + NQ], ot[:]
                    )

            def attention_head(h, emit_oproj=False):
                h2, hp = h // 2, 64 * (h % 2)
                hpS = 64 - hp  # partition base of head h in the swapped copies
                for j in range(QTILES):
                    av = psum.tile([65, NQ], f32, tag="av", bufs=2)
                    ktiles = 4 * j + 4 if causal else KVTILES

                    def off_of(t):
                        d = t - 4 * j
                        return 128 * d if (causal and d >= 0) else 0

                    if OPT_SCADJ:
                        groups = [(2 * u, 2 * u + 1) for u in range(ktiles // 2)]
                    else:
                        groups = [(t,) for t in range(ktiles)]
                    for ts in groups:
                        sps, pps = [], []
                        # adjacent score matmuls at complementary row halves ->
                        # they run concurrently in the PE array
                        for t in ts:
                            off = off_of(t)
                            if t % 2 == 0:
                                kp, qp, base = kproj, qproj, hp
                            else:
                                kp, qp, base = kprojS, qprojS, hpS
                            sp = psum.tile([128, NQ], f32, tag="sc", bufs=3)
                            nc.tensor.matmul(
                                sp[:, off:NQ],
                                kp[base : base + 64, h2, 128 * t : 128 * t + 128],
                                qp[base : base + 64, h2, NQ * j + off : NQ * j + NQ],
                                start=True,
                                stop=True,
                            )
                            sps.append(sp)
                        for t, sp in zip(ts, sps):
                            off = off_of(t)
                            p = pbuf.tile([128, NQ], bf16, tag="p")
                            nc.scalar.activation(p[:, off:NQ], sp[:, off:NQ], Exp)
                            pps.append(p)
                        for t, p in zip(ts, pps):
                            off = off_of(t)
                            d = t - 4 * j
                            if causal:
                                if d >= 0:
                                    nc.vector.tensor_mul(
                                        p[:, off : off + 128],
                                        p[:, off : off + 128],
                                        mask128[:],
                                    )
                            else:
                                mt_t = mpool.tile([128, NQ], bf16, tag="mt")
                                nc.sync.dma_start(
                                    mt_t[:],
                                    maskT[128 * t : 128 * t + 128, NQ * j : NQ * j + NQ],
                                )
                                nc.vector.tensor_mul(p[:], p[:], mt_t[:])
                        for t, p in zip(ts, pps):
                            off = off_of(t)
                            nc.tensor.matmul(
                                av[:, off:NQ],
                                vproj[:, t, h, 1:66],
                                p[:, off:NQ],
                                start=(t == 0),
                                stop=(t == ktiles - 1),
                            )
                    # normalize: attn2[0:64, h, q] = av[0:64, q] / av[64, q]
                    rs = rpool.tile([65, NQ], f32, tag="rs")
                    nc.vector.tensor_copy(rs[64:65, :], av[64:65, :])
                    rq = rpool.tile([128, 4], f32, tag="rq")
                    nc.sync.dma_start(rq[:], rs[64:65, :])
                    rqr = rpool.tile([128, 4], f32, tag="rqr")
                    nc.vector.reciprocal(rqr[:], rq[:])
                    rr = rpool.tile([1, NQ], f32, tag="rr")
                    nc.sync.dma_start(rr[:], rqr[:])
                    rb = rpool.tile([64, NQ], f32, tag="rb")
                    nc.gpsimd.partition_broadcast(rb[:], rr[0:1, :], channels=64)
                    if h % 2 == 0:
                        nc.vector.tensor_mul(
                            attn2[0:64, h // 2, NQ * j : NQ * j + NQ], av[0:64, :], rb[:]
                        )
                    else:
                        tmpn = rpool.tile([64, NQ], bf16, tag="tmpn")
                        nc.vector.tensor_mul(tmpn[:], av[0:64, :], rb[:])
                        nc.sync.dma_start(
                            attn2[64:128, h // 2, NQ * j : NQ * j + NQ], tmpn[:]
                        )
                    if emit_oproj:
                        oproj_groups(j)

            attention_head(0)
            attention_head(1)
            attention_head(2)
            attention_head(3, emit_oproj=OPT_INLINE)
            if not OPT_INLINE:
                for n in range(QTILES):
                    oproj_groups(n)

    nc.compile()
    return nc


def _get_program(causal: bool):
    if causal not in _programs:
        _programs[causal] = _build_program(causal)
    return _programs[causal]


def kernel(query, key, value, mask, Wq, Wk, Wv, Wo):
    global last_results
    from concourse.bass_utils import run_bass_kernel_spmd

    query = np.asarray(query, dtype=np.float32)
    key = np.asarray(key, dtype=np.float32)
    value = np.asarray(value, dtype=np.float32)
    Wq = np.asarray(Wq, dtype=np.float32)
    Wk = np.asarray(Wk, dtype=np.float32)
    Wv = np.asarray(Wv, dtype=np.float32)
    Wo = np.asarray(Wo, dtype=np.float32)
    m2d = np.asarray(mask).reshape(S, S).astype(bool)

    causal = bool(np.array_equal(m2d, np.tril(np.ones((S, S), dtype=bool))))
    nc = _get_program(causal)

    scale = 1.0 / math.sqrt(DH)
    WqT = np.ascontiguousarray((Wq * scale).T).astype(_BF16)
    WkT = np.ascontiguousarray(Wk.T).astype(_BF16)
    WvT = np.ascontiguousarray(Wv.T).astype(_BF16)
    WoT = np.ascontiguousarray(Wo.T).astype(_BF16)
    xT = {
        "qT": [np.ascontiguousarray(query[b].T).astype(_BF16) for b in range(B)],
        "kT": [np.ascontiguousarray(key[b].T).astype(_BF16) for b in range(B)],
        "vT": [np.ascontiguousarray(value[b].T).astype(_BF16) for b in range(B)],
    }
    if not causal:
        maskT = np.ascontiguousarray(m2d.T).astype(_BF16)

    in_maps = []
    for c in range(NCORES):
        b, g = c // 4, c % 4
        sl = slice(256 * g, 256 * g + 256)
        im = {
            "qT": xT["qT"][b],
            "kT": xT["kT"][b],
            "vT": xT["vT"][b],
            "wqT": np.ascontiguousarray(WqT[:, sl]),
            "wkT": np.ascontiguousarray(WkT[:, sl]),
            "wvT": np.ascontiguousarray(WvT[:, sl]),
            "woT": np.ascontiguousarray(WoT[sl, :]),
        }
        if not causal:
            im["maskT"] = maskT
        in_maps.append(im)

    trace = os.environ.get("KERNEL_PROFILE", "") == "1"
    res = run_bass_kernel_spmd(nc, in_maps, list(range(NCORES)), trace=trace)
    last_results = res

    outp = np.empty((B, S, D), dtype=np.float32)
    for b in range(B):
        acc = res.results[4 * b]["out"].astype(np.float32)
        for g in range(1, 4):
            acc = acc + res.results[4 * b + g]["out"]
        outp[b] = acc.T
    return outp
